# revision 23
# baseline (speedup 1.0000x reference)
"""GCEncoder (RGCN basis-decomposition conv + mean aggregation + Dense/BN/ReLU)
as a Bass/Tile kernel on 8 Trainium2 NeuronCores.

Math (reference):
  W[r]  = sum_b comp[r,b] * basis[b]                    [R, N, H0]
  h[r]  = x @ W[r]                                      [R, N, H0]
  agg[d] = sum_r (1/cnt[d,r]) * sum_{e: dst=d, type=r} h[r, src_e]
  feats = agg + x @ root + bias
  z     = feats @ fc_w.T ; per-row batchnorm over H1 + gamma/beta + relu
  out   = (z[:U], z[U:]) stacked -> [2, U, H1]

Everything before the BN is linear in the H0 axis, so fc_w is folded into
the weights on the host: W'[r] = W[r] @ fc_w.T (4096 x 75), root' =
root @ fc_w.T, bias' = bias @ fc_w.T.  The device only moves 75-wide
features:

  z[d] = sum_{r,s} ATw[(r,s), d] * h'_r[s] + x[d] @ root' + bias'

with ATw the host-built normalized adjacency (1/cnt[d,r] baked in, bf16)
and h'_r = x @ W'_r.  ~6.7x fewer device FLOPs than the unfolded form.

Device strategy (per core c of 8, 512 node-rows each):
  warmup: a dummy 8-byte AllGather first thing absorbs the one-time
          ~20us CC-engine warmup off the critical path.
  Phase A: h'|root'-part = x[rows] @ [W'_0|..|W'_4|root'] (4096 x 450),
           loaded in 8 fine-grained tile pairs so the first matmul
           starts as soon as the first ~1MB lands.  root' block stays
           in fp32 SBUF; h' block -> bf16 SBUF tiles (these ARE the
           local phase-B stationary operands) and one 384KB AllGather.
  Phase B: single PSUM accumulation [75, 512]: 20 local k-tiles run
           during the AllGather (h' straight from SBUF), then 140
           remote k-tiles (gathered h' stationary, ATw tiles moving,
           512-wide streams).
  Phase C: 4 PE transposes -> [dst, 75]; + root' + bias'; per-row BN
           (bn_stats/bn_aggr) + gamma/beta + ReLU.
"""
import numpy as np
import ml_dtypes

import concourse.bacc as bacc
import concourse.mybir as mybir
import concourse.tile as tile
from concourse.bass_utils import run_bass_kernel_spmd
from concourse.masks import make_identity

P = 128
NCORES = 8
N = 4096          # nodes
U = 2048          # users
R = 5             # relations
H0 = 500
H1 = 75
EPS = 1e-5

NL = N // NCORES              # 512 node rows per core
KB_A = N // P                 # 32 contraction tiles, phase A
GB_A = 16                     # phase-A load groups
KPG = KB_A // GB_A            # 4 kb per group
WCOL = R * H1 + H1            # 450 folded-weight columns
MB = NL // P                  # 4 M-tiles per core
KT_B = R * MB                 # 20 k-tiles per (core-block) in phase B

F32 = mybir.dt.float32
BF16 = mybir.dt.bfloat16

# test hooks
TRACE = False
LAST_RESULTS = None
_NC_CACHE = None


def _build():
    nc = bacc.Bacc("TRN2", target_bir_lowering=False, debug=False,
                   num_devices=NCORES)

    # host-swizzled inputs; layouts noted as [partition, free...]
    # x4[p, kb*NL + m] = x[coreRows m][i = kb*128+p]
    x4_d = nc.dram_tensor("x4", [P, KB_A * NL], BF16, kind="ExternalInput")
    # w4[p, kb*WCOL + j] = Wall'[kb*128+p, j]
    w4_d = nc.dram_tensor("w4", [P, KB_A * WCOL], BF16, kind="ExternalInput")
    # at4[p, t*NL + d] = ATw[(r, src), myDst d]; t = (cb, r, mk),
    # src = cb*512 + mk*128 + p
    at4_d = nc.dram_tensor("at4", [P, NCORES * KT_B * NL], BF16,
                           kind="ExternalInput")
    biasb_d = nc.dram_tensor("biasb", [P, H1], F32, kind="ExternalInput")
    gamma_d = nc.dram_tensor("gamma", [P, MB], F32, kind="ExternalInput")
    beta_d = nc.dram_tensor("beta", [P, MB], F32, kind="ExternalInput")
    out_d = nc.dram_tensor("out", [NL, H1], F32, kind="ExternalOutput")

    with tile.TileContext(nc) as tc:
        with (
            tc.tile_pool(name="big", bufs=1) as big,
            tc.tile_pool(name="xtp", bufs=GB_A) as xtp,
            tc.tile_pool(name="wtp", bufs=GB_A) as wtp,
            tc.tile_pool(name="io", bufs=4) as iop,
            tc.tile_pool(name="hhp", bufs=4) as hhp,
            tc.tile_pool(name="atp", bufs=3) as atp,
            tc.tile_pool(name="persist", bufs=4) as pp,
            tc.tile_pool(name="bn", bufs=4) as bnp,
            tc.tile_pool(name="ps", bufs=8, space="PSUM") as psp,
            tc.tile_pool(name="dram", bufs=1, space="DRAM") as dramp,
        ):
            # ---------------- Phase A: h' = x_rows @ Wall' ----------------
            xg, wg = [], []
            for g in range(GB_A):
                xt = xtp.tile([P, KPG, NL], BF16, tag="xt", name=f"xt_{g}")
                nc.sync.dma_start(
                    out=xt, in_=x4_d[:, g * KPG * NL:(g + 1) * KPG * NL])
                xg.append(xt)
                wt = wtp.tile([P, KPG, WCOL], BF16, tag="wt", name=f"wt_{g}")
                nc.scalar.dma_start(
                    out=wt, in_=w4_d[:, g * KPG * WCOL:(g + 1) * KPG * WCOL])
                wg.append(wt)

            ps_m = [psp.tile([P, WCOL], F32, tag="ps", name=f"psA_{m}")
                    for m in range(MB)]
            for g in range(GB_A):
                for kb in range(KPG):
                    for m in range(MB):
                        nc.tensor.matmul(
                            ps_m[m],
                            xg[g][:, kb, m * P:(m + 1) * P],
                            wg[g][:, kb, :],
                            start=(g == 0 and kb == 0),
                            stop=(g == GB_A - 1 and kb == KPG - 1),
                        )

            # h_cr[p, m*375 + r*75 + j] = h'[m*128+p, r*75+j]
            h_cr = dramp.tile([P, MB * R * H1], BF16, tag="h_c")
            h_ar = dramp.tile([NCORES * P, MB * R * H1], BF16, tag="h_a",
                              addr_space="Shared")
            rootf, hb16 = [], []
            for m in range(MB):
                rf = pp.tile([P, H1], F32, tag="rootf", name=f"rootf_{m}")
                nc.vector.tensor_copy(out=rf, in_=ps_m[m][:, R * H1:])
                rootf.append(rf)
                hb = iop.tile([P, R * H1], BF16, tag="hout", name=f"hout_{m}")
                nc.vector.tensor_copy(out=hb, in_=ps_m[m][:, :R * H1])
                hb16.append(hb)
                nc.gpsimd.dma_start(
                    out=h_cr[:, m * R * H1:(m + 1) * R * H1], in_=hb)
            nc.gpsimd.collective_compute(
                "AllGather",
                mybir.AluOpType.bypass,
                replica_groups=[list(range(NCORES))],
                ins=[h_cr[:, :]],
                outs=[h_ar[:, :]],
            )

            # ------- Phase B: S = sum_(r,s) h'-tile.T @ ATw-tile ----------
            # 4 interleaved PSUM accumulation chains hide per-matmul
            # overhead (phase A's 4-chain ILP measured 217ns vs 339ns for
            # v2's single-chain B); combined on DVE afterwards.
            NCH = 4
            psS = [psp.tile([H1, NL], F32, tag="ps", name=f"psB_{ch}")
                   for ch in range(NCH)]
            NT_B = NCORES * KT_B
            for cb in range(NCORES):
                hh = hhp.tile([P, MB * R * H1], BF16, tag="hh",
                              name=f"hh_{cb}")
                nc.gpsimd.dma_start(out=hh, in_=h_ar[cb * P:(cb + 1) * P, :])
                aa = atp.tile([P, KT_B, NL], BF16, tag="aa", name=f"aa_{cb}")
                base = cb * KT_B * NL
                for q in range(4):
                    if cb < 3:
                        # gate the 21MB at4 stream behind phase A's last
                        # loads so it can't steal HBM bandwidth from the
                        # A-critical x/W tiles (WAW orders the DMA after
                        # this copy, which reads the last x tile)
                        nc.vector.tensor_copy(
                            out=aa[0:1, q * (KT_B // 4), 0:1],
                            in_=xg[GB_A - 1][0:1, 0, 0:1])
                    nc.scalar.dma_start(
                        out=aa[:, q * (KT_B // 4):(q + 1) * (KT_B // 4), :],
                        in_=at4_d[:, base + q * (KT_B // 4) * NL:
                                  base + (q + 1) * (KT_B // 4) * NL])
                for r in range(R):
                    for mk in range(MB):
                        t = cb * KT_B + r * MB + mk
                        nc.tensor.matmul(
                            psS[t % NCH],
                            hh[:, mk * R * H1 + r * H1:
                               mk * R * H1 + (r + 1) * H1],
                            aa[:, r * MB + mk, :],
                            start=(t < NCH),
                            stop=(t >= NT_B - NCH),
                        )

            # ---------------- Phase C: combine -> BN -> ReLU --------------
            ident = big.tile([P, P], F32, tag="ident")
            make_identity(nc, ident)
            biasb = big.tile([P, H1], F32, tag="bias")
            nc.scalar.dma_start(out=biasb, in_=biasb_d[:, :])
            gam = big.tile([P, MB], F32, tag="gam")
            nc.scalar.dma_start(out=gam, in_=gamma_d[:, :])
            bet = big.tile([P, MB], F32, tag="bet")
            nc.scalar.dma_start(out=bet, in_=beta_d[:, :])
            eps_t = big.tile([P, 1], F32, tag="eps")
            nc.vector.memset(eps_t, EPS)

            sT = pp.tile([H1, NL], F32, tag="sT")
            nc.vector.tensor_copy(out=sT, in_=psS[0])
            for ch in range(1, 4):
                nc.vector.tensor_add(out=sT, in0=sT, in1=psS[ch])

            for m in range(MB):
                pt = psp.tile([P, H1], F32, tag="ps", name=f"pt_{m}")
                nc.tensor.transpose(
                    pt, sT[:, m * P:(m + 1) * P], ident[:H1, :H1]
                )
                acc = bnp.tile([P, H1], F32, tag="acc", name=f"acc_{m}")
                nc.vector.tensor_add(out=acc, in0=pt, in1=rootf[m])
                nc.vector.tensor_add(out=acc, in0=acc, in1=biasb)

                stats = bnp.tile([P, 6], F32, tag="stats")
                nc.vector.bn_stats(out=stats, in_=acc)
                mv = bnp.tile([P, 2], F32, tag="mv")
                nc.vector.bn_aggr(out=mv, in_=stats)
                rstd = bnp.tile([P, 1], F32, tag="rstd")
                nc.scalar.activation(
                    out=rstd, in_=mv[:, 1:2],
                    func=mybir.ActivationFunctionType.Sqrt,
                    bias=eps_t, scale=1.0,
                )
                nc.vector.reciprocal(out=rstd, in_=rstd)
                g2 = bnp.tile([P, 1], F32, tag="g2")
                nc.vector.tensor_mul(out=g2, in0=rstd, in1=gam[:, m:m + 1])
                zt = bnp.tile([P, H1], F32, tag="zt")
                nc.vector.tensor_scalar(
                    out=zt, in0=acc,
                    scalar1=mv[:, 0:1], scalar2=g2,
                    op0=mybir.AluOpType.subtract, op1=mybir.AluOpType.mult,
                )
                nc.scalar.activation(
                    out=zt, in_=zt,
                    func=mybir.ActivationFunctionType.Relu,
                    bias=bet[:, m:m + 1], scale=1.0,
                )
                nc.scalar.dma_start(out=out_d[m * P:(m + 1) * P, :], in_=zt)

    nc.finalize()
    return nc


def _get_nc():
    global _NC_CACHE
    if _NC_CACHE is None:
        _NC_CACHE = _build()
    return _NC_CACHE


def kernel(**inputs) -> np.ndarray:
    global LAST_RESULTS
    x = np.asarray(inputs["x"], dtype=np.float32)
    basis = np.asarray(inputs["basis"], dtype=np.float32)
    comp = np.asarray(inputs["comp"], dtype=np.float32)
    root = np.asarray(inputs["root"], dtype=np.float32)
    bias_rgcn = np.asarray(inputs["bias_rgcn"], dtype=np.float32)
    fc_w = np.asarray(inputs["fc_w"], dtype=np.float32)
    bn_gamma_u = np.asarray(inputs["bn_gamma_u"], dtype=np.float32)
    bn_beta_u = np.asarray(inputs["bn_beta_u"], dtype=np.float32)
    bn_gamma_i = np.asarray(inputs["bn_gamma_i"], dtype=np.float32)
    bn_beta_i = np.asarray(inputs["bn_beta_i"], dtype=np.float32)
    edge_index = np.asarray(inputs["edge_index"]).astype(np.int64)
    edge_type = np.asarray(inputs["edge_type"]).astype(np.int64)

    src, dst = edge_index[0], edge_index[1]
    et = edge_type

    # Wall' = [W_r @ fc_w.T for r | root @ fc_w.T]  (fold the Dense layer)
    W = np.tensordot(comp, basis, axes=([1], [0]))          # [R, N, H0]
    Wp = np.einsum("rio,jo->rij", W, fc_w, optimize=True)   # [R, N, H1]
    wall = np.empty((N, WCOL), dtype=np.float32)
    wall[:, :R * H1] = Wp.transpose(1, 0, 2).reshape(N, R * H1)
    wall[:, R * H1:] = root @ fc_w.T
    w4 = np.ascontiguousarray(
        wall.astype(ml_dtypes.bfloat16)
        .reshape(KB_A, P, WCOL)                 # [kb, p, j]
        .transpose(1, 0, 2)                     # [p, kb, j]
        .reshape(P, KB_A * WCOL))

    xT16 = np.ascontiguousarray(x.T).astype(ml_dtypes.bfloat16)
    # x4[p, kb*NL + m] = x.T[kb*128+p, m@core]  (per-core slice below)
    x4_full = (xT16.reshape(KB_A, P, N)         # [kb, p, s]
               .transpose(1, 0, 2))             # [p, kb, s]

    # normalized adjacency: ATw[(r, src), dst] = multiplicity / cnt[dst, r]
    cnt = np.bincount(dst * R + et, minlength=N * R).astype(np.float64)
    w_e = 1.0 / np.maximum(cnt[dst * R + et], 1.0)
    lin = (et * N + src) * np.int64(N) + dst
    atw = np.bincount(lin, weights=w_e, minlength=R * N * N)
    atw = atw.astype(ml_dtypes.bfloat16).reshape(R, NCORES, MB, P, N)

    biasb = np.ascontiguousarray(
        np.broadcast_to(bias_rgcn @ fc_w.T, (P, H1)), dtype=np.float32)
    gamma_all = np.concatenate([bn_gamma_u, bn_gamma_i])
    beta_all = np.concatenate([bn_beta_u, bn_beta_i])

    in_maps = []
    for c in range(NCORES):
        sl = slice(c * NL, (c + 1) * NL)
        atc = atw[:, :, :, :, sl]               # [r, cb, mk, p, d]
        at4 = atc.transpose(3, 1, 0, 2, 4).reshape(P, NCORES * KT_B * NL)
        in_maps.append({
            "x4": np.ascontiguousarray(
                x4_full[:, :, sl]).reshape(P, KB_A * NL),
            "w4": w4,
            "at4": np.ascontiguousarray(at4),
            "biasb": biasb,
            "gamma": np.ascontiguousarray(gamma_all[sl].reshape(MB, P).T),
            "beta": np.ascontiguousarray(beta_all[sl].reshape(MB, P).T),
        })

    nc = _get_nc()
    res = run_bass_kernel_spmd(
        nc, in_maps, core_ids=list(range(NCORES)), trace=TRACE,
    )
    LAST_RESULTS = res

    z = np.concatenate([res.results[c]["out"] for c in range(NCORES)], axis=0)
    return np.stack([z[:U], z[U:]], axis=0)


# revision 25
# speedup vs baseline: 1.0181x; 1.0181x over previous
"""GCEncoder (RGCN basis-decomposition conv + mean aggregation + Dense/BN/ReLU)
as a Bass/Tile kernel on 8 Trainium2 NeuronCores.

Math (reference):
  W[r]  = sum_b comp[r,b] * basis[b]                    [R, N, H0]
  h[r]  = x @ W[r]                                      [R, N, H0]
  agg[d] = sum_r (1/cnt[d,r]) * sum_{e: dst=d, type=r} h[r, src_e]
  feats = agg + x @ root + bias
  z     = feats @ fc_w.T ; per-row batchnorm over H1 + gamma/beta + relu
  out   = (z[:U], z[U:]) stacked -> [2, U, H1]

Everything before the BN is linear in the H0 axis, so fc_w is folded into
the weights on the host: W'[r] = W[r] @ fc_w.T (4096 x 75), root' =
root @ fc_w.T, bias' = bias @ fc_w.T.  The device only moves 75-wide
features (~6.7x fewer matmul FLOPs than the unfolded form):

  z[d] = sum_r (1/cnt[d,r]) * (Mcnt_r[d,:] @ h'_r) + x[d] @ root' + bias'

with Mcnt_r the integer edge-multiplicity matrix (exact in fp8e4m3, half
the HBM bytes of a bf16 weighted adjacency) and h'_r = x @ W'_r.

Scheduling facts learned from perfetto traces on this stack:
  - the collective trigger waits for ALL in-flight hardware-DGE DMA to
    drain, so bulk loads must either complete before the A->AllGather
    handoff or ride the software-DGE (gpsimd) queue, which is exempt;
  - a collective costs ~11us trigger->mesh plus ~28ns/KB, so ONE
    AllGather of all 375 h' columns beats per-relation chunking;
  - interleaved PSUM accumulation chains hide the per-matmul weight
    load (phase A: 4 banks; phase B: the 5 relation groups, mk-inner).

Device strategy (per core c of 8, 512 node-rows each):
  Phase A: h'|root' = x[rows] @ [W'_0|..|W'_4|root'] (4096 x 450) in 16
           fine-grained load groups (x on sync-DGE, W' on scalar-DGE),
           m-tiles in 2 pair-passes (3 PSUM banks).  root' block stays
           in fp32 SBUF; h' -> bf16 SBUF -> h_cr DRAM -> one 384KB
           AllGather (issued from the gpsimd queue).
  Phase B: per relation r: S_r[75, 512] accumulated over 32 src-tiles
           (stationary bf16 h' tiles, moving fp8 count tiles, 512-wide
           streams; 5 PSUM banks, r-inner order for 5-way chain ILP).
           Count tiles: 3 chunks prefetched during A on scalar-DGE, the
           rest streamed post-AllGather on gpsimd.
  Phase C: per (m,r): PE-transpose S_r -> [dst,75], scale by
           cinv[d,r] (tensor_scalar), accumulate; + root' + bias';
           per-row BN (bn_stats/bn_aggr) + gamma/beta + ReLU.
"""
import numpy as np
import ml_dtypes

import concourse.bacc as bacc
import concourse.mybir as mybir
import concourse.tile as tile
from concourse.bass_utils import run_bass_kernel_spmd
from concourse.masks import make_identity

P = 128
NCORES = 8
N = 4096          # nodes
U = 2048          # users
R = 5             # relations
H0 = 500
H1 = 75
EPS = 1e-5

NL = N // NCORES              # 512 node rows per core
KB_A = N // P                 # 32 contraction tiles, phase A
GB_A = 16                     # phase-A load groups
KPG = KB_A // GB_A            # 2 kb per group
WCOL = R * H1 + H1            # 450 folded-weight columns
MB = NL // P                  # 4 M-tiles per core
KT_B = R * MB                 # 20 k-tiles per core-block in phase B
WARM = 3                      # at4 chunks prefetched during phase A

F32 = mybir.dt.float32
BF16 = mybir.dt.bfloat16
FP8 = mybir.dt.float8e4
NP_FP8 = ml_dtypes.float8_e4m3

# test hooks
TRACE = False
LAST_RESULTS = None
_NC_CACHE = None


def _build():
    nc = bacc.Bacc("TRN2", target_bir_lowering=False, debug=False,
                   num_devices=NCORES)

    # host-swizzled inputs; layouts noted as [partition, free...]
    # x4[p, kb*NL + m] = x[coreRows m][i = kb*128+p]
    x4_d = nc.dram_tensor("x4", [P, KB_A * NL], BF16, kind="ExternalInput")
    # w4[p, kb*WCOL + j] = Wall'[kb*128+p, j]
    w4_d = nc.dram_tensor("w4", [P, KB_A * WCOL], BF16, kind="ExternalInput")
    # at4[p, t*NL + d] = Mcnt[(r, src), myDst d]; t = (cb, r, mk),
    # src = cb*512 + mk*128 + p  (fp8 integer counts)
    at4_d = nc.dram_tensor("at4", [P, NCORES * KT_B * NL], FP8,
                           kind="ExternalInput")
    # cinv[p, m*R + r] = 1 / max(cnt[dst = m*128+p @ core, r], 1)
    cinv_d = nc.dram_tensor("cinv", [P, MB * R], F32, kind="ExternalInput")
    biasb_d = nc.dram_tensor("biasb", [P, H1], F32, kind="ExternalInput")
    gamma_d = nc.dram_tensor("gamma", [P, MB], F32, kind="ExternalInput")
    beta_d = nc.dram_tensor("beta", [P, MB], F32, kind="ExternalInput")
    out_d = nc.dram_tensor("out", [NL, H1], F32, kind="ExternalOutput")

    with tile.TileContext(nc) as tc:
        with (
            tc.tile_pool(name="big", bufs=1) as big,
            tc.tile_pool(name="xtp", bufs=GB_A) as xtp,
            tc.tile_pool(name="wtp", bufs=GB_A) as wtp,
            tc.tile_pool(name="io", bufs=4) as iop,
            tc.tile_pool(name="hhp", bufs=4) as hhp,
            tc.tile_pool(name="atp", bufs=WARM) as atp,
            tc.tile_pool(name="persist", bufs=4) as pp,
            tc.tile_pool(name="stp", bufs=5) as stp,
            tc.tile_pool(name="bn", bufs=4) as bnp,
            tc.tile_pool(name="psA", bufs=3, space="PSUM") as psa,
            tc.tile_pool(name="psB", bufs=5, space="PSUM") as psb,
            tc.tile_pool(name="dram", bufs=1, space="DRAM") as dramp,
        ):
            # ---------------- Phase A: h' = x_rows @ Wall' ----------------
            xg, wg = [], []
            for g in range(GB_A):
                xt = xtp.tile([P, KPG, NL], BF16, tag="xt", name=f"xt_{g}")
                nc.sync.dma_start(
                    out=xt, in_=x4_d[:, g * KPG * NL:(g + 1) * KPG * NL])
                xg.append(xt)
                wt = wtp.tile([P, KPG, WCOL], BF16, tag="wt", name=f"wt_{g}")
                nc.scalar.dma_start(
                    out=wt, in_=w4_d[:, g * KPG * WCOL:(g + 1) * KPG * WCOL])
                wg.append(wt)

            # h_cr[p, m*375 + r*75 + j] = h'[m*128+p, r*75+j]
            h_cr = dramp.tile([P, MB * R * H1], BF16, tag="h_c")
            h_ar = dramp.tile([NCORES * P, MB * R * H1], BF16, tag="h_a",
                              addr_space="Shared")
            rootf, hb16 = [], []
            for mg in range(2):          # m-pairs: 2-bank ILP, 3 psA bufs
                ps_m = [psa.tile([P, WCOL], F32, tag="psA",
                                 name=f"psA_{mg}_{mi}") for mi in range(2)]
                for g in range(GB_A):
                    for kb in range(KPG):
                        for mi in range(2):
                            m = mg * 2 + mi
                            nc.tensor.matmul(
                                ps_m[mi],
                                xg[g][:, kb, m * P:(m + 1) * P],
                                wg[g][:, kb, :],
                                start=(g == 0 and kb == 0),
                                stop=(g == GB_A - 1 and kb == KPG - 1),
                            )
                for mi in range(2):
                    m = mg * 2 + mi
                    rf = pp.tile([P, H1], F32, tag="rootf", name=f"rootf_{m}")
                    nc.vector.tensor_copy(out=rf, in_=ps_m[mi][:, R * H1:])
                    rootf.append(rf)
                    hb = iop.tile([P, R * H1], BF16, tag="hout",
                                  name=f"hout_{m}")
                    nc.vector.tensor_copy(out=hb, in_=ps_m[mi][:, :R * H1])
                    hb16.append(hb)
                    nc.gpsimd.dma_start(
                        out=h_cr[:, m * R * H1:(m + 1) * R * H1], in_=hb)
            nc.gpsimd.collective_compute(
                "AllGather",
                mybir.AluOpType.bypass,
                replica_groups=[list(range(NCORES))],
                ins=[h_cr[:, :]],
                outs=[h_ar[:, :]],
            )

            # ------- Phase B: S_r = sum_s h'_r-tile.T @ Mcnt-tile ---------
            psS = [psb.tile([H1, NL], F32, tag="psB", name=f"psB_{r}")
                   for r in range(R)]
            for cb in range(NCORES):
                hh = hhp.tile([P, MB * R * H1], BF16, tag="hh",
                              name=f"hh_{cb}")
                nc.gpsimd.dma_start(out=hh, in_=h_ar[cb * P:(cb + 1) * P, :])
                aa = atp.tile([P, KT_B, NL], FP8, tag="aa", name=f"aa_{cb}")
                base = cb * KT_B * NL
                if cb < WARM:
                    # prefetched during phase A on scalar hardware-DGE;
                    # drained well before the collective trigger
                    for q in range(2):
                        nc.scalar.dma_start(
                            out=aa[:, q * (KT_B // 2):(q + 1) * (KT_B // 2),
                                   :],
                            in_=at4_d[:, base + q * (KT_B // 2) * NL:
                                      base + (q + 1) * (KT_B // 2) * NL])
                else:
                    # streamed post-AllGather on the software-DGE queue
                    # (exempt from the collective's DMA-drain barrier)
                    nc.gpsimd.dma_start(out=aa, in_=at4_d[:, base:
                                                          base + KT_B * NL])
                # r-inner: consecutive matmuls hit 5 different PSUM banks
                for mk in range(MB):
                    for r in range(R):
                        nc.tensor.matmul(
                            psS[r],
                            hh[:, mk * R * H1 + r * H1:
                               mk * R * H1 + (r + 1) * H1],
                            aa[:, r * MB + mk, :],
                            start=(cb == 0 and mk == 0),
                            stop=(cb == NCORES - 1 and mk == MB - 1),
                        )

            # ---------------- Phase C: combine -> BN -> ReLU --------------
            ident = big.tile([P, P], F32, tag="ident")
            make_identity(nc, ident)
            biasb = big.tile([P, H1], F32, tag="bias")
            nc.scalar.dma_start(out=biasb, in_=biasb_d[:, :])
            cinv = big.tile([P, MB * R], F32, tag="cinv")
            nc.scalar.dma_start(out=cinv, in_=cinv_d[:, :])
            gam = big.tile([P, MB], F32, tag="gam")
            nc.scalar.dma_start(out=gam, in_=gamma_d[:, :])
            bet = big.tile([P, MB], F32, tag="bet")
            nc.scalar.dma_start(out=bet, in_=beta_d[:, :])
            eps_t = big.tile([P, 1], F32, tag="eps")
            nc.vector.memset(eps_t, EPS)

            sT = []
            for r in range(R):
                st = stp.tile([H1, NL], F32, tag="sT", name=f"sT_{r}")
                nc.vector.tensor_copy(out=st, in_=psS[r])
                sT.append(st)

            for m in range(MB):
                acc = bnp.tile([P, H1], F32, tag="acc", name=f"acc_{m}")
                nc.vector.tensor_add(out=acc, in0=rootf[m], in1=biasb)
                for r in range(R):
                    pt = psa.tile([P, H1], F32, tag="psA",
                                  name=f"pt_{m}_{r}")
                    nc.tensor.transpose(
                        pt, sT[r][:, m * P:(m + 1) * P], ident[:H1, :H1]
                    )
                    sc = bnp.tile([P, H1], F32, tag="sc")
                    nc.vector.tensor_scalar(
                        out=sc, in0=pt,
                        scalar1=cinv[:, m * R + r:m * R + r + 1], scalar2=None,
                        op0=mybir.AluOpType.mult,
                    )
                    nc.vector.tensor_add(out=acc, in0=acc, in1=sc)

                stats = bnp.tile([P, 6], F32, tag="stats")
                nc.vector.bn_stats(out=stats, in_=acc)
                mv = bnp.tile([P, 2], F32, tag="mv")
                nc.vector.bn_aggr(out=mv, in_=stats)
                rstd = bnp.tile([P, 1], F32, tag="rstd")
                nc.scalar.activation(
                    out=rstd, in_=mv[:, 1:2],
                    func=mybir.ActivationFunctionType.Sqrt,
                    bias=eps_t, scale=1.0,
                )
                nc.vector.reciprocal(out=rstd, in_=rstd)
                g2 = bnp.tile([P, 1], F32, tag="g2")
                nc.vector.tensor_mul(out=g2, in0=rstd, in1=gam[:, m:m + 1])
                zt = bnp.tile([P, H1], F32, tag="zt")
                nc.vector.tensor_scalar(
                    out=zt, in0=acc,
                    scalar1=mv[:, 0:1], scalar2=g2,
                    op0=mybir.AluOpType.subtract, op1=mybir.AluOpType.mult,
                )
                nc.scalar.activation(
                    out=zt, in_=zt,
                    func=mybir.ActivationFunctionType.Relu,
                    bias=bet[:, m:m + 1], scale=1.0,
                )
                nc.scalar.dma_start(out=out_d[m * P:(m + 1) * P, :], in_=zt)

    nc.finalize()
    return nc


def _get_nc():
    global _NC_CACHE
    if _NC_CACHE is None:
        _NC_CACHE = _build()
    return _NC_CACHE


def kernel(**inputs) -> np.ndarray:
    global LAST_RESULTS
    x = np.asarray(inputs["x"], dtype=np.float32)
    basis = np.asarray(inputs["basis"], dtype=np.float32)
    comp = np.asarray(inputs["comp"], dtype=np.float32)
    root = np.asarray(inputs["root"], dtype=np.float32)
    bias_rgcn = np.asarray(inputs["bias_rgcn"], dtype=np.float32)
    fc_w = np.asarray(inputs["fc_w"], dtype=np.float32)
    bn_gamma_u = np.asarray(inputs["bn_gamma_u"], dtype=np.float32)
    bn_beta_u = np.asarray(inputs["bn_beta_u"], dtype=np.float32)
    bn_gamma_i = np.asarray(inputs["bn_gamma_i"], dtype=np.float32)
    bn_beta_i = np.asarray(inputs["bn_beta_i"], dtype=np.float32)
    edge_index = np.asarray(inputs["edge_index"]).astype(np.int64)
    edge_type = np.asarray(inputs["edge_type"]).astype(np.int64)

    src, dst = edge_index[0], edge_index[1]
    et = edge_type

    # Wall' = [W_r @ fc_w.T for r | root @ fc_w.T]  (fold the Dense layer)
    W = np.tensordot(comp, basis, axes=([1], [0]))          # [R, N, H0]
    Wp = np.einsum("rio,jo->rij", W, fc_w, optimize=True)   # [R, N, H1]
    wall = np.empty((N, WCOL), dtype=np.float32)
    wall[:, :R * H1] = Wp.transpose(1, 0, 2).reshape(N, R * H1)
    wall[:, R * H1:] = root @ fc_w.T
    w4 = np.ascontiguousarray(
        wall.astype(ml_dtypes.bfloat16)
        .reshape(KB_A, P, WCOL)                 # [kb, p, j]
        .transpose(1, 0, 2)                     # [p, kb, j]
        .reshape(P, KB_A * WCOL))

    xT16 = np.ascontiguousarray(x.T).astype(ml_dtypes.bfloat16)
    # x4[p, kb*NL + m] = x.T[kb*128+p, m@core]  (per-core slice below)
    x4_full = (xT16.reshape(KB_A, P, N)         # [kb, p, s]
               .transpose(1, 0, 2))             # [p, kb, s]

    # integer multiplicity matrix Mcnt[(r, src), dst] (exact in fp8e4m3)
    lin = (et * N + src) * np.int64(N) + dst
    cntmat = np.bincount(lin, minlength=R * N * N)
    assert cntmat.max() <= 16, "edge multiplicity too large for fp8 counts"
    atw = cntmat.astype(NP_FP8).reshape(R, NCORES, MB, P, N)

    # per-(dst, r) inverse counts
    cnt = np.bincount(dst * R + et, minlength=N * R).astype(np.float64)
    cinv_full = (1.0 / np.maximum(cnt, 1.0)).astype(np.float32).reshape(N, R)

    biasb = np.ascontiguousarray(
        np.broadcast_to(bias_rgcn @ fc_w.T, (P, H1)), dtype=np.float32)
    gamma_all = np.concatenate([bn_gamma_u, bn_gamma_i])
    beta_all = np.concatenate([bn_beta_u, bn_beta_i])

    in_maps = []
    for c in range(NCORES):
        sl = slice(c * NL, (c + 1) * NL)
        atc = atw[:, :, :, :, sl]               # [r, cb, mk, p, d]
        at4 = atc.transpose(3, 1, 0, 2, 4).reshape(P, NCORES * KT_B * NL)
        cinv_c = cinv_full[sl].reshape(MB, P, R).transpose(1, 0, 2)
        in_maps.append({
            "x4": np.ascontiguousarray(
                x4_full[:, :, sl]).reshape(P, KB_A * NL),
            "w4": w4,
            "at4": np.ascontiguousarray(at4),
            "cinv": np.ascontiguousarray(cinv_c.reshape(P, MB * R)),
            "biasb": biasb,
            "gamma": np.ascontiguousarray(gamma_all[sl].reshape(MB, P).T),
            "beta": np.ascontiguousarray(beta_all[sl].reshape(MB, P).T),
        })

    nc = _get_nc()
    res = run_bass_kernel_spmd(
        nc, in_maps, core_ids=list(range(NCORES)), trace=TRACE,
    )
    LAST_RESULTS = res

    z = np.concatenate([res.results[c]["out"] for c in range(NCORES)], axis=0)
    return np.stack([z[:U], z[U:]], axis=0)
